# revision 6
# baseline (speedup 1.0000x reference)
"""DWT (db4, depthwise stride-2) layer as a Trainium2 Bass/Tile kernel.

Math: for input x[B, T, C] and 8-tap filters lo/hi, the reference computes a
reflect-pad-7 depthwise stride-2 cross-correlation cropped by 3 on each side:

    out[b, t', c]     = sum_k lo[k] * xe[b, 2 t' + k, c]
    out[b, t', C + c] = sum_k hi[k] * xe[b, 2 t' + k, c]

where xe[u] = x[u - 1] for u in [1, T+1), xe[0] = x[1], xe[T+1] = x[T-2]
(only 1 reflected element is needed on each side after the crop), and
t' in [0, T/2 - 2).

Mapping to the PE: put time on the partition axis, 2 consecutive time steps
per partition (so DMA partition lines are 512 B), and reduce over partitions
with banded stationary matrices (polyphase decomposition):

    X[p, b, j, c] = xe[u0 + 2 p + j, ...]            (j in {0,1})
    psum_f[m, b, c] = sum_p  W_f_j0[p, m] X[p,b,0,c] + W_f_j1[p, m] X[p,b,1,c]
    W_f_j[p, m] = f[2 (p - m) + j]  for p - m in [0, 4), else 0

giving 125 outputs t' = u0/2 + m per 128-partition tile. The two filter
outputs are interleaved into [t', b, (lo|hi) c] in SBUF so the store DMA is
fully contiguous per (b, t') (512 B chunks, contiguous across t').

Sharding: data-parallel over batch, 4 batches per core on 8 cores.
"""

import math

import numpy as np

import concourse.bacc as bacc
import concourse.mybir as mybir
import concourse.tile as tile
from concourse.bass_utils import run_bass_kernel_spmd

F32 = mybir.dt.float32
F32R = mybir.dt.float32r

B, T, C = 32, 16384, 64
N_CORES = 8
BL = B // N_CORES  # 4 batches per core
KTAPS = 8
M = 125  # output positions per 128-partition tile


def _build_nc(Bl: int, Tn: int, Cn: int, mm_dtype=F32):
    """Build + compile the single-core Bass program for x[Bl, Tn, Cn]."""
    nout = Tn // 2 - 2
    ntiles = math.ceil(nout / M)
    nfree = Bl * Cn  # moving free dim of each matmul

    nc = bacc.Bacc("TRN2", target_bir_lowering=False, debug=False)
    x_d = nc.dram_tensor("x", [Bl, Tn, Cn], F32, kind="ExternalInput")
    w_d = nc.dram_tensor("w", [4, 128, M], F32, kind="ExternalInput")
    o_d = nc.dram_tensor("out", [Bl, nout, 2 * Cn], F32, kind="ExternalOutput")

    with tile.TileContext(nc) as tc:
        with (
            tc.tile_pool(name="wpool", bufs=1) as wpool,
            tc.tile_pool(name="xin", bufs=4) as xpool,
            tc.tile_pool(name="oout", bufs=3) as opool,
            tc.tile_pool(name="ps", bufs=4, space="PSUM") as pspool,
        ):
            # Stationary banded matrices: [lo_j0, lo_j1, hi_j0, hi_j1]
            w_t = wpool.tile([128, 4 * M], F32)
            nc.sync.dma_start(out=w_t[:].rearrange("p (f m) -> p f m", f=4),
                              in_=w_d.rearrange("f p m -> p f m"))
            def _mm_view(ap):
                return ap if mm_dtype == F32 else ap.bitcast(mm_dtype)

            w_ap = [_mm_view(w_t[:, f * M:(f + 1) * M]) for f in range(4)]

            for i in range(ntiles):
                t0 = M * i
                nvalid = min(M, nout - t0)
                u0 = 2 * t0

                xt = xpool.tile([128, Bl * 2 * Cn], F32)
                xv = xt[:].rearrange("p (b j c) -> p b j c", b=Bl, j=2)
                if i == 0:
                    # p = 0: j=0 <- x[1] (reflected), j=1 <- x[0]
                    nc.sync.dma_start(out=xv[0:1, :, 0, :],
                                      in_=x_d[:, 1:2, :].rearrange("b t c -> t b c"))
                    nc.sync.dma_start(out=xv[0:1, :, 1, :],
                                      in_=x_d[:, 0:1, :].rearrange("b t c -> t b c"))
                    nc.sync.dma_start(
                        out=xt[1:128],
                        in_=x_d[:, 1:255, :].rearrange("b (p w) c -> p b (w c)", w=2))
                elif i == ntiles - 1:
                    pfull = (Tn - u0) // 2  # partitions fully served by real x data
                    if pfull + 1 < 128:
                        # zero the tail partitions; the DMAs below overwrite
                        # the real rows (DVE memset must start at partition 0)
                        nc.vector.memset(xt[:], 0.0)
                    nc.sync.dma_start(
                        out=xt[0:pfull],
                        in_=x_d[:, u0 - 1:u0 - 1 + 2 * pfull, :]
                        .rearrange("b (p w) c -> p b (w c)", w=2))
                    # tail partition: j=0 <- x[T-1], j=1 <- x[T-2] (reflected)
                    nc.sync.dma_start(out=xv[pfull:pfull + 1, :, 0, :],
                                      in_=x_d[:, Tn - 1:Tn, :].rearrange("b t c -> t b c"))
                    nc.sync.dma_start(out=xv[pfull:pfull + 1, :, 1, :],
                                      in_=x_d[:, Tn - 2:Tn - 1, :].rearrange("b t c -> t b c"))
                else:
                    nc.sync.dma_start(
                        out=xt[:],
                        in_=x_d[:, u0 - 1:u0 + 255, :]
                        .rearrange("b (p w) c -> p b (w c)", w=2))

                x0 = _mm_view(xv[:, :, 0, :])
                x1 = _mm_view(xv[:, :, 1, :])
                ps_lo = pspool.tile([M, nfree], F32, tag="ps")
                ps_hi = pspool.tile([M, nfree], F32, tag="ps")
                nc.tensor.matmul(out=ps_lo[:], lhsT=w_ap[0], rhs=x0, start=True, stop=False)
                nc.tensor.matmul(out=ps_lo[:], lhsT=w_ap[1], rhs=x1, start=False, stop=True)
                nc.tensor.matmul(out=ps_hi[:], lhsT=w_ap[2], rhs=x0, start=True, stop=False)
                nc.tensor.matmul(out=ps_hi[:], lhsT=w_ap[3], rhs=x1, start=False, stop=True)

                ot = opool.tile([128, Bl * 2 * Cn], F32)
                ov = ot[:].rearrange("p (b f c) -> p b f c", b=Bl, f=2)
                nc.vector.tensor_copy(
                    out=ov[0:nvalid, :, 0, :],
                    in_=ps_lo[0:nvalid].rearrange("p (b c) -> p b c", b=Bl))
                nc.vector.tensor_copy(
                    out=ov[0:nvalid, :, 1, :],
                    in_=ps_hi[0:nvalid].rearrange("p (b c) -> p b c", b=Bl))
                nc.sync.dma_start(
                    out=o_d[:, t0:t0 + nvalid, :].rearrange("b t c -> t b c"),
                    in_=ot[0:nvalid].rearrange("p (b w) -> p b w", b=Bl))

    nc.compile()
    return nc


def _build_w(dec_lo: np.ndarray, dec_hi: np.ndarray) -> np.ndarray:
    """Banded stationary matrices [4, 128, M]: order lo_j0, lo_j1, hi_j0, hi_j1."""
    lo = np.asarray(dec_lo, np.float32)
    hi = np.asarray(dec_hi, np.float32)
    w = np.zeros((4, 128, M), np.float32)
    for m in range(M):
        for d in range(4):
            w[0, m + d, m] = lo[2 * d]
            w[1, m + d, m] = lo[2 * d + 1]
            w[2, m + d, m] = hi[2 * d]
            w[3, m + d, m] = hi[2 * d + 1]
    return w


_NC_CACHE = {}


def _get_nc():
    key = (BL, T, C)
    if key not in _NC_CACHE:
        _NC_CACHE[key] = _build_nc(*key)
    return _NC_CACHE[key]


def kernel(x: np.ndarray, dec_lo: np.ndarray, dec_hi: np.ndarray) -> np.ndarray:
    x = np.asarray(x, np.float32)
    assert x.shape == (B, T, C), x.shape
    nc = _get_nc()
    w = _build_w(dec_lo, dec_hi)
    in_maps = [
        {"x": np.ascontiguousarray(x[i * BL:(i + 1) * BL]), "w": w}
        for i in range(N_CORES)
    ]
    res = run_bass_kernel_spmd(nc, in_maps, core_ids=list(range(N_CORES)))
    return np.concatenate([res.results[i]["out"] for i in range(N_CORES)], axis=0)


# revision 8
# speedup vs baseline: 1.0026x; 1.0026x over previous
"""DWT (db4, depthwise stride-2) layer as a Trainium2 Bass/Tile kernel.

Math: for input x[B, T, C] and 8-tap filters lo/hi, the reference computes a
reflect-pad-7 depthwise stride-2 cross-correlation cropped by 3 on each side:

    out[b, t', c]     = sum_k lo[k] * xe[b, 2 t' + k, c]
    out[b, t', C + c] = sum_k hi[k] * xe[b, 2 t' + k, c]

where xe[u] = x[u - 1] for u in [1, T+1), xe[0] = x[1], xe[T+1] = x[T-2]
(only 1 reflected element is needed on each side after the crop), and
t' in [0, T/2 - 2).

Mapping to the PE: put time on the partition axis, 2 consecutive time steps
per partition (so DMA partition lines are 512 B), and reduce over partitions
with banded stationary matrices (polyphase decomposition):

    X[p, b, j, c] = xe[u0 + 2 p + j, ...]            (j in {0,1})
    psum_f[m, b, c] = sum_p  W_f_j0[p, m] X[p,b,0,c] + W_f_j1[p, m] X[p,b,1,c]
    W_f_j[p, m] = f[2 (p - m) + j]  for p - m in [0, 4), else 0

giving 125 outputs t' = u0/2 + m per 128-partition tile. The two filter
outputs are interleaved into [t', b, (lo|hi) c] in SBUF so the store DMA is
fully contiguous per (b, t') (512 B chunks, contiguous across t').

Sharding: data-parallel over batch, 4 batches per core on 8 cores.
"""

import math

import numpy as np

import concourse.bacc as bacc
import concourse.mybir as mybir
import concourse.tile as tile
from concourse.bass_utils import run_bass_kernel_spmd

F32 = mybir.dt.float32
F32R = mybir.dt.float32r

B, T, C = 32, 16384, 64
N_CORES = 8
BL = B // N_CORES  # 4 batches per core
KTAPS = 8
M = 125  # output positions per 128-partition tile


def _build_nc(Bl: int, Tn: int, Cn: int, mm_dtype=F32):
    """Build + compile the single-core Bass program for x[Bl, Tn, Cn]."""
    nout = Tn // 2 - 2
    ntiles = math.ceil(nout / M)
    nfree = Bl * Cn  # moving free dim of each matmul

    nc = bacc.Bacc("TRN2", target_bir_lowering=False, debug=False)
    x_d = nc.dram_tensor("x", [Bl, Tn, Cn], F32, kind="ExternalInput")
    w_d = nc.dram_tensor("w", [4, 128, M], F32, kind="ExternalInput")
    o_d = nc.dram_tensor("out", [Bl, nout, 2 * Cn], F32, kind="ExternalOutput")

    with tile.TileContext(nc) as tc:
        with (
            tc.tile_pool(name="wpool", bufs=1) as wpool,
            tc.tile_pool(name="xin", bufs=6) as xpool,
            tc.tile_pool(name="oout", bufs=4) as opool,
            tc.tile_pool(name="ps", bufs=4, space="PSUM") as pspool,
        ):
            # Stationary banded matrices: [lo_j0, lo_j1, hi_j0, hi_j1]
            w_t = wpool.tile([128, 4 * M], F32)
            nc.sync.dma_start(out=w_t[:].rearrange("p (f m) -> p f m", f=4),
                              in_=w_d.rearrange("f p m -> p f m"))
            def _mm_view(ap):
                return ap if mm_dtype == F32 else ap.bitcast(mm_dtype)

            w_ap = [_mm_view(w_t[:, f * M:(f + 1) * M]) for f in range(4)]

            for i in range(ntiles):
                t0 = M * i
                nvalid = min(M, nout - t0)
                u0 = 2 * t0

                xt = xpool.tile([128, Bl * 2 * Cn], F32)
                xv = xt[:].rearrange("p (b j c) -> p b j c", b=Bl, j=2)
                if i == 0:
                    # p = 0: j=0 <- x[1] (reflected), j=1 <- x[0]
                    nc.sync.dma_start(out=xv[0:1, :, 0, :],
                                      in_=x_d[:, 1:2, :].rearrange("b t c -> t b c"))
                    nc.sync.dma_start(out=xv[0:1, :, 1, :],
                                      in_=x_d[:, 0:1, :].rearrange("b t c -> t b c"))
                    nc.sync.dma_start(
                        out=xt[1:128],
                        in_=x_d[:, 1:255, :].rearrange("b (p w) c -> p b (w c)", w=2))
                elif i == ntiles - 1:
                    pfull = (Tn - u0) // 2  # partitions fully served by real x data
                    if pfull + 1 < 128:
                        # zero the tail partitions; the DMAs below overwrite
                        # the real rows (DVE memset must start at partition 0)
                        nc.vector.memset(xt[:], 0.0)
                    nc.sync.dma_start(
                        out=xt[0:pfull],
                        in_=x_d[:, u0 - 1:u0 - 1 + 2 * pfull, :]
                        .rearrange("b (p w) c -> p b (w c)", w=2))
                    # tail partition: j=0 <- x[T-1], j=1 <- x[T-2] (reflected)
                    nc.sync.dma_start(out=xv[pfull:pfull + 1, :, 0, :],
                                      in_=x_d[:, Tn - 1:Tn, :].rearrange("b t c -> t b c"))
                    nc.sync.dma_start(out=xv[pfull:pfull + 1, :, 1, :],
                                      in_=x_d[:, Tn - 2:Tn - 1, :].rearrange("b t c -> t b c"))
                else:
                    nc.sync.dma_start(
                        out=xt[:],
                        in_=x_d[:, u0 - 1:u0 + 255, :]
                        .rearrange("b (p w) c -> p b (w c)", w=2))

                x0 = _mm_view(xv[:, :, 0, :])
                x1 = _mm_view(xv[:, :, 1, :])
                ps_lo = pspool.tile([M, nfree], F32, tag="ps")
                ps_hi = pspool.tile([M, nfree], F32, tag="ps")
                nc.tensor.matmul(out=ps_lo[:], lhsT=w_ap[0], rhs=x0, start=True, stop=False)
                nc.tensor.matmul(out=ps_lo[:], lhsT=w_ap[1], rhs=x1, start=False, stop=True)
                nc.tensor.matmul(out=ps_hi[:], lhsT=w_ap[2], rhs=x0, start=True, stop=False)
                nc.tensor.matmul(out=ps_hi[:], lhsT=w_ap[3], rhs=x1, start=False, stop=True)

                ot = opool.tile([128, Bl * 2 * Cn], F32)
                ov = ot[:].rearrange("p (b f c) -> p b f c", b=Bl, f=2)
                nc.vector.tensor_copy(
                    out=ov[0:nvalid, :, 0, :],
                    in_=ps_lo[0:nvalid].rearrange("p (b c) -> p b c", b=Bl))
                nc.vector.tensor_copy(
                    out=ov[0:nvalid, :, 1, :],
                    in_=ps_hi[0:nvalid].rearrange("p (b c) -> p b c", b=Bl))
                # per-batch stores (contiguous 64 KB DRAM region each),
                # alternating HWDGE rings to spread SDMA engine load
                for b in range(Bl):
                    eng = nc.sync if b % 2 == 0 else nc.scalar
                    eng.dma_start(
                        out=o_d[b, t0:t0 + nvalid, :],
                        in_=ot[0:nvalid, b * 2 * Cn:(b + 1) * 2 * Cn])

    nc.compile()
    return nc


def _build_w(dec_lo: np.ndarray, dec_hi: np.ndarray) -> np.ndarray:
    """Banded stationary matrices [4, 128, M]: order lo_j0, lo_j1, hi_j0, hi_j1."""
    lo = np.asarray(dec_lo, np.float32)
    hi = np.asarray(dec_hi, np.float32)
    w = np.zeros((4, 128, M), np.float32)
    for m in range(M):
        for d in range(4):
            w[0, m + d, m] = lo[2 * d]
            w[1, m + d, m] = lo[2 * d + 1]
            w[2, m + d, m] = hi[2 * d]
            w[3, m + d, m] = hi[2 * d + 1]
    return w


_NC_CACHE = {}


def _get_nc():
    key = (BL, T, C)
    if key not in _NC_CACHE:
        _NC_CACHE[key] = _build_nc(*key)
    return _NC_CACHE[key]


def kernel(x: np.ndarray, dec_lo: np.ndarray, dec_hi: np.ndarray) -> np.ndarray:
    x = np.asarray(x, np.float32)
    assert x.shape == (B, T, C), x.shape
    nc = _get_nc()
    w = _build_w(dec_lo, dec_hi)
    in_maps = [
        {"x": np.ascontiguousarray(x[i * BL:(i + 1) * BL]), "w": w}
        for i in range(N_CORES)
    ]
    res = run_bass_kernel_spmd(nc, in_maps, core_ids=list(range(N_CORES)))
    return np.concatenate([res.results[i]["out"] for i in range(N_CORES)], axis=0)


# revision 9
# speedup vs baseline: 1.0082x; 1.0055x over previous
"""DWT (db4, depthwise stride-2) layer as a Trainium2 Bass/Tile kernel.

Math: for input x[B, T, C] and 8-tap filters lo/hi, the reference computes a
reflect-pad-7 depthwise stride-2 cross-correlation cropped by 3 on each side:

    out[b, t', c]     = sum_k lo[k] * xe[b, 2 t' + k, c]
    out[b, t', C + c] = sum_k hi[k] * xe[b, 2 t' + k, c]

where xe[u] = x[u - 1] for u in [1, T+1), xe[0] = x[1], xe[T+1] = x[T-2]
(only 1 reflected element is needed on each side after the crop), and
t' in [0, T/2 - 2).

Mapping to the PE: put time on the partition axis, 2 consecutive time steps
per partition (so DMA partition lines are 512 B), and reduce over partitions
with banded stationary matrices (polyphase decomposition):

    X[p, b, j, c] = xe[u0 + 2 p + j, ...]            (j in {0,1})
    psum_f[m, b, c] = sum_p  W_f_j0[p, m] X[p,b,0,c] + W_f_j1[p, m] X[p,b,1,c]
    W_f_j[p, m] = f[2 (p - m) + j]  for p - m in [0, 4), else 0

giving 125 outputs t' = u0/2 + m per 128-partition tile. The two filter
outputs are interleaved into [t', b, (lo|hi) c] in SBUF so the store DMA is
fully contiguous per (b, t') (512 B chunks, contiguous across t').

Sharding: data-parallel over batch, 4 batches per core on 8 cores.
"""

import math

import numpy as np

import concourse.bacc as bacc
import concourse.mybir as mybir
import concourse.tile as tile
from concourse.bass_utils import run_bass_kernel_spmd

F32 = mybir.dt.float32
F32R = mybir.dt.float32r

B, T, C = 32, 16384, 64
N_CORES = 8
BL = B // N_CORES  # 4 batches per core
KTAPS = 8
M = 125  # output positions per 128-partition tile


def _build_nc(Bl: int, Tn: int, Cn: int, mm_dtype=F32):
    """Build + compile the single-core Bass program for x[Bl, Tn, Cn]."""
    nout = Tn // 2 - 2
    ntiles = math.ceil(nout / M)
    nfree = Bl * Cn  # moving free dim of each matmul

    nc = bacc.Bacc("TRN2", target_bir_lowering=False, debug=False)
    x_d = nc.dram_tensor("x", [Bl, Tn, Cn], F32, kind="ExternalInput")
    w_d = nc.dram_tensor("w", [4, 128, M], F32, kind="ExternalInput")
    o_d = nc.dram_tensor("out", [Bl, nout, 2 * Cn], F32, kind="ExternalOutput")

    with tile.TileContext(nc) as tc:
        with (
            tc.tile_pool(name="wpool", bufs=1) as wpool,
            tc.tile_pool(name="xin", bufs=6) as xpool,
            tc.tile_pool(name="oout", bufs=4) as opool,
            tc.tile_pool(name="ps", bufs=4, space="PSUM") as pspool,
        ):
            # Stationary banded matrices: [lo_j0, lo_j1, hi_j0, hi_j1]
            w_t = wpool.tile([128, 4 * M], F32)
            nc.sync.dma_start(out=w_t[:].rearrange("p (f m) -> p f m", f=4),
                              in_=w_d.rearrange("f p m -> p f m"))
            def _mm_view(ap):
                return ap if mm_dtype == F32 else ap.bitcast(mm_dtype)

            w_ap = [_mm_view(w_t[:, f * M:(f + 1) * M]) for f in range(4)]

            for i in range(ntiles):
                t0 = M * i
                nvalid = min(M, nout - t0)
                u0 = 2 * t0

                xt = xpool.tile([128, Bl * 2 * Cn], F32)
                xv = xt[:].rearrange("p (b j c) -> p b j c", b=Bl, j=2)
                if i == 0:
                    # p = 0: j=0 <- x[1] (reflected), j=1 <- x[0]
                    nc.sync.dma_start(out=xv[0:1, :, 0, :],
                                      in_=x_d[:, 1:2, :].rearrange("b t c -> t b c"))
                    nc.sync.dma_start(out=xv[0:1, :, 1, :],
                                      in_=x_d[:, 0:1, :].rearrange("b t c -> t b c"))
                    nc.sync.dma_start(
                        out=xt[1:128],
                        in_=x_d[:, 1:255, :].rearrange("b (p w) c -> p b (w c)", w=2))
                elif i == ntiles - 1:
                    pfull = (Tn - u0) // 2  # partitions fully served by real x data
                    if pfull + 1 < 128:
                        # zero the tail partitions; the DMAs below overwrite
                        # the real rows (DVE memset must start at partition 0)
                        nc.vector.memset(xt[:], 0.0)
                    nc.sync.dma_start(
                        out=xt[0:pfull],
                        in_=x_d[:, u0 - 1:u0 - 1 + 2 * pfull, :]
                        .rearrange("b (p w) c -> p b (w c)", w=2))
                    # tail partition: j=0 <- x[T-1], j=1 <- x[T-2] (reflected)
                    nc.sync.dma_start(out=xv[pfull:pfull + 1, :, 0, :],
                                      in_=x_d[:, Tn - 1:Tn, :].rearrange("b t c -> t b c"))
                    nc.sync.dma_start(out=xv[pfull:pfull + 1, :, 1, :],
                                      in_=x_d[:, Tn - 2:Tn - 1, :].rearrange("b t c -> t b c"))
                else:
                    nc.sync.dma_start(
                        out=xt[:],
                        in_=x_d[:, u0 - 1:u0 + 255, :]
                        .rearrange("b (p w) c -> p b (w c)", w=2))

                x0 = _mm_view(xv[:, :, 0, :])
                x1 = _mm_view(xv[:, :, 1, :])
                ps_lo = pspool.tile([M, nfree], F32, tag="ps")
                ps_hi = pspool.tile([M, nfree], F32, tag="ps")
                nc.tensor.matmul(out=ps_lo[:], lhsT=w_ap[0], rhs=x0, start=True, stop=False)
                nc.tensor.matmul(out=ps_lo[:], lhsT=w_ap[1], rhs=x1, start=False, stop=True)
                nc.tensor.matmul(out=ps_hi[:], lhsT=w_ap[2], rhs=x0, start=True, stop=False)
                nc.tensor.matmul(out=ps_hi[:], lhsT=w_ap[3], rhs=x1, start=False, stop=True)

                ot = opool.tile([128, Bl * 2 * Cn], F32)
                ov = ot[:].rearrange("p (b f c) -> p b f c", b=Bl, f=2)
                nc.vector.tensor_copy(
                    out=ov[0:nvalid, :, 0, :],
                    in_=ps_lo[0:nvalid].rearrange("p (b c) -> p b c", b=Bl))
                nc.vector.tensor_copy(
                    out=ov[0:nvalid, :, 1, :],
                    in_=ps_hi[0:nvalid].rearrange("p (b c) -> p b c", b=Bl))
                # per-batch stores (contiguous 64 KB DRAM region each).
                # SWDGE (gpsimd): HWDGE store descriptors all land on a
                # fixed 5-engine SDMA subset; the SWDGE ring spreads wider.
                for b in range(Bl):
                    nc.gpsimd.dma_start(
                        out=o_d[b, t0:t0 + nvalid, :],
                        in_=ot[0:nvalid, b * 2 * Cn:(b + 1) * 2 * Cn])

    nc.compile()
    return nc


def _build_w(dec_lo: np.ndarray, dec_hi: np.ndarray) -> np.ndarray:
    """Banded stationary matrices [4, 128, M]: order lo_j0, lo_j1, hi_j0, hi_j1."""
    lo = np.asarray(dec_lo, np.float32)
    hi = np.asarray(dec_hi, np.float32)
    w = np.zeros((4, 128, M), np.float32)
    for m in range(M):
        for d in range(4):
            w[0, m + d, m] = lo[2 * d]
            w[1, m + d, m] = lo[2 * d + 1]
            w[2, m + d, m] = hi[2 * d]
            w[3, m + d, m] = hi[2 * d + 1]
    return w


_NC_CACHE = {}


def _get_nc():
    key = (BL, T, C)
    if key not in _NC_CACHE:
        _NC_CACHE[key] = _build_nc(*key)
    return _NC_CACHE[key]


def kernel(x: np.ndarray, dec_lo: np.ndarray, dec_hi: np.ndarray) -> np.ndarray:
    x = np.asarray(x, np.float32)
    assert x.shape == (B, T, C), x.shape
    nc = _get_nc()
    w = _build_w(dec_lo, dec_hi)
    in_maps = [
        {"x": np.ascontiguousarray(x[i * BL:(i + 1) * BL]), "w": w}
        for i in range(N_CORES)
    ]
    res = run_bass_kernel_spmd(nc, in_maps, core_ids=list(range(N_CORES)))
    return np.concatenate([res.results[i]["out"] for i in range(N_CORES)], axis=0)
